# revision 14
# baseline (speedup 1.0000x reference)
"""Trainium2 Bass kernel for nn_AttentionBlock (B=2, L=2048, D=1024, H=16).

Sharding: tensor-parallel over heads. Each of 8 cores computes 2 heads:
Wq/Wk/Wv column-sharded, Wo row-sharded; host sums the 8 partial outputs.

Per-core dataflow (all matmuls fp32r = full-rate fp32 with 11-bit mantissa):
  - x^T is prepared host-side ([D, B*L], layout prep only, no math).
  - qT/kT/vT = W.T @ xT   (weight-stationary, contraction over D)
  - vT is PE-transposed to v [L, dh] with a ones column appended (aug),
    so the PV matmul also produces the softmax denominators for free.
  - scoresT = kT.T @ qT per (head, batch) in [Lk, Lq] layout; exp on ACT
    (no max-subtraction: scores ~ N(0,1), exp is fp32-safe);
  - aT += v_aug.T @ expT accumulates attention output (and denominator row).
  - aT is normalized in-place via a PE-broadcast reciprocal matrix.
  - out = aT.T @ Wo (heads accumulate in PSUM), written as [B*L, D] partial.
"""
import numpy as np
from contextlib import ExitStack

import concourse.bacc as bacc
import concourse.tile as tile
import concourse.mybir as mybir
from concourse import bass_utils
from concourse.masks import make_identity

F32 = mybir.dt.float32
F32R = mybir.dt.float32r
AF = mybir.ActivationFunctionType
ALU = mybir.AluOpType

B, L, D, H, DH = 2, 2048, 1024, 16, 64
NCORES = 8
HPC = H // NCORES       # heads per core
DHC = HPC * DH          # 128 = head-dim slice per core
KT = D // 128           # 8 k-tiles over the contraction dim


def build(Lb=L, debug=False, use_tilepos=True, use_rep=True):
    """Build the per-core Bass program for per-batch seq len Lb."""
    BLb = B * Lb
    NJT = Lb // 128            # key tiles per batch
    LC = min(512, Lb)          # query-chunk width
    NLC = Lb // LC             # query chunks per batch
    PC = min(512, BLb)         # projection chunk width
    NPC = BLb // PC            # projection chunks
    VB = 130                   # v block width per (b, jt): 2 heads x (64+ones)

    nc = bacc.Bacc("TRN2", target_bir_lowering=False, debug=debug, num_devices=8)

    xT = nc.dram_tensor("xT", [D, BLb], F32R, kind="ExternalInput")
    wq = nc.dram_tensor("wq", [D, DHC], F32R, kind="ExternalInput")
    wk = nc.dram_tensor("wk", [D, DHC], F32R, kind="ExternalInput")
    wv = nc.dram_tensor("wv", [D, DHC], F32R, kind="ExternalInput")
    wo = nc.dram_tensor("wo", [DHC, D], F32R, kind="ExternalInput")
    bq = nc.dram_tensor("bq", [DHC, 1], F32, kind="ExternalInput")
    bk = nc.dram_tensor("bk", [DHC, 1], F32, kind="ExternalInput")
    bv = nc.dram_tensor("bv", [DHC, 1], F32, kind="ExternalInput")
    out = nc.dram_tensor("out", [BLb, D], F32, kind="ExternalOutput")

    xT_v = xT.ap().rearrange("(kt p) l -> p kt l", p=128)   # [128, KT, BLb]
    wq_v = wq.ap().rearrange("(kt p) m -> p kt m", p=128)   # [128, KT, DHC]
    wk_v = wk.ap().rearrange("(kt p) m -> p kt m", p=128)
    wv_v = wv.ap().rearrange("(kt p) m -> p kt m", p=128)

    with tile.TileContext(nc) as tc, ExitStack() as ctx:
        # --- pools ---
        persist = ctx.enter_context(tc.tile_pool(name="persist", bufs=1))
        xpool = ctx.enter_context(tc.tile_pool(name="xchunk", bufs=3))
        vstage = ctx.enter_context(tc.tile_pool(name="vstage", bufs=2))
        expool = ctx.enter_context(tc.tile_pool(name="expool", bufs=4))
        denpool = ctx.enter_context(tc.tile_pool(name="denpool", bufs=2))
        outpool = ctx.enter_context(tc.tile_pool(name="outpool", bufs=3))
        # PSUM budget: "pair" = 2 banks/tile x3 + "single" = 1 bank/tile x2 -> 8
        scpool = ctx.enter_context(tc.tile_pool(name="scpool", bufs=2, space="PSUM"))
        accpool = ctx.enter_context(tc.tile_pool(name="accpool", bufs=1, space="PSUM"))
        psing = ctx.enter_context(tc.tile_pool(name="psing", bufs=2, space="PSUM"))

        # --- persistent tiles ---
        qT_sb = persist.tile([128, BLb], F32R, tag="qT")
        kT_sb = persist.tile([128, BLb], F32R, tag="kT")
        v_sb = persist.tile([128, B * NJT * VB], F32R, tag="v")
        aT_sb = [
            persist.tile([128, Lb], F32R, tag=f"aT{b}", name=f"aT{b}")
            for b in range(B)
        ]
        wq_sb = persist.tile([128, KT, DHC], F32R, tag="wq")
        wk_sb = persist.tile([128, KT, DHC], F32R, tag="wk")
        wv_sb = persist.tile([128, KT, DHC], F32R, tag="wv")
        wo_sb = persist.tile([DHC, D], F32R, tag="wo")
        bq_sb = persist.tile([DHC, 1], F32, tag="bq")
        bk_sb = persist.tile([DHC, 1], F32, tag="bk")
        bv_sb = persist.tile([DHC, 1], F32, tag="bv")
        ident = persist.tile([128, 128], F32, tag="ident")
        ones64 = persist.tile([65, 64], F32, tag="ones64")  # row 64 used

        # --- phase A: loads & constants ---
        nc.sync.dma_start(wq_sb[:], wq_v)
        nc.sync.dma_start(wk_sb[:], wk_v)
        nc.sync.dma_start(wv_sb[:], wv_v)
        nc.sync.dma_start(wo_sb[:], wo.ap())
        nc.sync.dma_start(bq_sb[:], bq.ap())
        nc.sync.dma_start(bk_sb[:], bk.ap())
        nc.sync.dma_start(bv_sb[:], bv.ap())
        make_identity(nc, ident[:])
        nc.vector.memset(ones64[:], 1.0)
        # fill the aug ones-columns of v (memset can't produce f32r)
        nblk = B * NJT * HPC
        ones_f = persist.tile([128, max(nblk, 64)], F32, tag="ones_f")
        nc.vector.memset(ones_f[:], 1.0)
        v_cols = v_sb[:].rearrange("p (n c) -> p n c", c=65)
        nc.vector.tensor_copy(
            v_cols[:, :, 64:65],
            ones_f[:, 0:nblk].rearrange("p (n c) -> p n c", c=1),
        )

        # --- phase B: projections (chunked over B*L) + v transpose ---
        for chn in range(NPC):
            cs = chn * PC
            xt = xpool.tile([128, KT, PC], F32R, tag="xt")
            nc.sync.dma_start(xt[:], xT_v[:, :, cs:cs + PC])

            for w_sb, b_sb, dst in ((wq_sb, bq_sb, qT_sb), (wk_sb, bk_sb, kT_sb)):
                ps = psing.tile([128, PC], F32, tag="single")
                for kt in range(KT):
                    nc.tensor.matmul(
                        ps[:, :], w_sb[:, kt, :], xt[:, kt, :],
                        start=(kt == 0), stop=(kt == KT - 1),
                    )
                nc.vector.tensor_scalar(
                    dst[:, cs:cs + PC], ps[:, :], b_sb[:, 0:1], None, ALU.add
                )

            # v chunk -> staging (vT layout), then transpose into v_sb blocks
            ps = psing.tile([128, PC], F32, tag="single")
            for kt in range(KT):
                nc.tensor.matmul(
                    ps[:, :], wv_sb[:, kt, :], xt[:, kt, :],
                    start=(kt == 0), stop=(kt == KT - 1),
                )
            vt = vstage.tile([128, PC], F32R, tag="vt")
            nc.vector.tensor_scalar(vt[:], ps[:, :], bv_sb[:, 0:1], None, ALU.add)

            for jl in range(PC // 128):
                gl = cs + jl * 128          # global row offset in [0, BLb)
                b_idx, jt = gl // Lb, (gl % Lb) // 128
                blk = (b_idx * NJT + jt) * VB
                pt = psing.tile([128, 128], F32, tag="single")
                # one [128,128] transpose covers both heads: out[j, h*64+d]
                nc.tensor.transpose(
                    pt[:, :],
                    vt[:, jl * 128:(jl + 1) * 128].bitcast(F32),
                    ident[:],
                )
                vdst = v_sb[:, blk:blk + VB].rearrange("p (h c) -> p h c", h=2)
                ptv = pt[:].rearrange("p (h c) -> p h c", h=2)
                nc.vector.tensor_copy(vdst[:, :, 0:DH], ptv[:, :, :])

        # --- phase C: attention per (batch, query-chunk) ---
        for b in range(B):
            for lc in range(NLC):
                q0 = b * Lb + lc * LC
                aT = accpool.tile([65, 2, LC], F32, tag="acc", padded_shape=[65, 2, 512])
                for jt in range(NJT):
                    k0 = b * Lb + jt * 128
                    blk = (b * NJT + jt) * VB
                    sc = scpool.tile([128, 2, LC], F32, tag="sc", padded_shape=[128, 2, 512])
                    for h in range(HPC):
                        nc.tensor.matmul(
                            sc[:, h, :],
                            kT_sb[h * DH:(h + 1) * DH, k0:k0 + 128],
                            qT_sb[h * DH:(h + 1) * DH, q0:q0 + LC],
                            start=True, stop=True,
                            tile_position=(h * DH, 0) if use_tilepos else None,
                        )
                    ex = expool.tile([128, 2, LC], F32R, tag="ex")
                    nc.scalar.activation(ex[:], sc[:], AF.Exp)
                    for h in range(HPC):
                        nc.tensor.matmul(
                            aT[:, h, :],
                            v_sb[:, blk + h * 65:blk + h * 65 + 65],
                            ex[:, h, :],
                            start=(jt == 0), stop=(jt == NJT - 1),
                        )
                # drain: copy heads into aT_sb; denominators -> reciprocal
                for h in range(HPC):
                    nc.vector.tensor_copy(
                        aT_sb[b][h * DH:(h + 1) * DH, lc * LC:lc * LC + LC],
                        aT[0:DH, h, :],
                    )
                den = denpool.tile([65, 2, LC], F32, tag="den")
                nc.vector.tensor_copy(den[64:65, :, :], aT[64:65, :, :])
                if use_rep:
                    # broadcast raw denominators across partitions, then ONE
                    # wide reciprocal (128 lanes) instead of a 1-lane recip
                    rep = psing.tile([128, LC], F32, tag="single")
                    for h in range(HPC):
                        nc.tensor.matmul(
                            rep[h * DH:(h + 1) * DH, :],
                            ones64[64:65, :],
                            den[64:65, h, :],
                            start=True, stop=True,
                            tile_position=(64, h * DH) if use_tilepos else None,
                        )
                    rrec = denpool.tile([128, LC], F32, tag="rrec")
                    nc.vector.reciprocal(rrec[:, :], rep[:, :])
                    nc.vector.tensor_mul(
                        aT_sb[b][:, lc * LC:lc * LC + LC],
                        aT_sb[b][:, lc * LC:lc * LC + LC],
                        rrec[:, :],
                    )
                # output projection for this query chunk
                for t in range(lc * LC // 128, (lc * LC + LC) // 128):
                    ot = outpool.tile([128, D], F32, tag="ot")
                    for nch in range(2):
                        po = psing.tile([128, 512], F32, tag="single")
                        nc.tensor.matmul(
                            po[:, :],
                            aT_sb[b][:, t * 128:(t + 1) * 128],
                            wo_sb[:, nch * 512:(nch + 1) * 512],
                            start=True, stop=True,
                        )
                        nc.vector.tensor_copy(
                            ot[:, nch * 512:(nch + 1) * 512], po[:, :]
                        )
                    nc.sync.dma_start(
                        out.ap()[b * Lb + t * 128:b * Lb + (t + 1) * 128, :], ot[:]
                    )


    nc.compile()
    return nc


_NC_CACHE = {}


def _get_nc(Lb=L):
    if Lb not in _NC_CACHE:
        _NC_CACHE[Lb] = build(Lb)
    return _NC_CACHE[Lb]


def make_in_maps(x, Wq, bq, Wk, bk, Wv, bv, Wo, bo, Lb=L):
    s = np.float32(DH ** (-0.25))
    BLb = B * Lb
    xT = np.ascontiguousarray(
        np.asarray(x, np.float32).reshape(BLb, D).T
    ).astype(np.float32)
    Wq, Wk, Wv, Wo = (np.asarray(a, np.float32) for a in (Wq, Wk, Wv, Wo))
    bq, bk, bv = (np.asarray(a, np.float32) for a in (bq, bk, bv))
    in_maps = []
    for c in range(NCORES):
        hs = slice(c * DHC, (c + 1) * DHC)
        in_maps.append({
            "xT": xT,
            "wq": np.ascontiguousarray(Wq[:, hs] * s),
            "wk": np.ascontiguousarray(Wk[:, hs] * s),
            "wv": np.ascontiguousarray(Wv[:, hs]),
            "wo": np.ascontiguousarray(Wo[hs, :]),
            "bq": np.ascontiguousarray((bq[hs] * s).reshape(DHC, 1)),
            "bk": np.ascontiguousarray((bk[hs] * s).reshape(DHC, 1)),
            "bv": np.ascontiguousarray(bv[hs].reshape(DHC, 1)),
        })
    return in_maps


def kernel(x, Wq, bq, Wk, bk, Wv, bv, Wo, bo, **run_kwargs):
    x = np.asarray(x, np.float32)
    nc = _get_nc(L)
    in_maps = make_in_maps(x, Wq, bq, Wk, bk, Wv, bv, Wo, bo, L)
    res = bass_utils.run_bass_kernel_spmd(nc, in_maps, list(range(NCORES)), **run_kwargs)
    acc = np.zeros((B * L, D), np.float32)
    for r in res.results:
        acc += r["out"]
    acc += np.asarray(bo, np.float32)[None, :]
    out = acc.reshape(B, L, D)
    kernel.last_results = res
    return out


# revision 15
# speedup vs baseline: 1.0621x; 1.0621x over previous
"""Trainium2 Bass kernel for nn_AttentionBlock (B=2, L=2048, D=1024, H=16).

Sharding: tensor-parallel over heads. Each of 8 cores computes 2 heads:
Wq/Wk/Wv column-sharded, Wo row-sharded; host sums the 8 partial outputs.

Per-core dataflow (all matmuls fp32r = full-rate fp32 with 11-bit mantissa):
  - x^T is prepared host-side ([D, B*L], layout prep only, no math).
  - qT/kT/vT = W.T @ xT   (weight-stationary, contraction over D)
  - vT is PE-transposed to v [L, dh] with a ones column appended (aug),
    so the PV matmul also produces the softmax denominators for free.
  - scoresT = kT.T @ qT per (head, batch) in [Lk, Lq] layout; exp on ACT
    (no max-subtraction: scores ~ N(0,1), exp is fp32-safe);
  - aT += v_aug.T @ expT accumulates attention output (and denominator row).
  - aT is normalized in-place via a PE-broadcast reciprocal matrix.
  - out = aT.T @ Wo (heads accumulate in PSUM), written as [B*L, D] partial.
"""
import numpy as np
from contextlib import ExitStack

import concourse.bacc as bacc
import concourse.tile as tile
import concourse.mybir as mybir
from concourse import bass_utils
from concourse.masks import make_identity

F32 = mybir.dt.float32
F32R = mybir.dt.float32r
AF = mybir.ActivationFunctionType
ALU = mybir.AluOpType

B, L, D, H, DH = 2, 2048, 1024, 16, 64
NCORES = 8
HPC = H // NCORES       # heads per core
DHC = HPC * DH          # 128 = head-dim slice per core
KT = D // 128           # 8 k-tiles over the contraction dim


def build(Lb=L, debug=False, use_tilepos=True, use_rep=True):
    """Build the per-core Bass program for per-batch seq len Lb."""
    BLb = B * Lb
    NJT = Lb // 128            # key tiles per batch
    LC = min(512, Lb)          # query-chunk width
    NLC = Lb // LC             # query chunks per batch
    PC = min(512, BLb)         # projection chunk width
    NPC = BLb // PC            # projection chunks
    VB = 130                   # v block width per (b, jt): 2 heads x (64+ones)

    nc = bacc.Bacc("TRN2", target_bir_lowering=False, debug=debug, num_devices=8)

    xT = nc.dram_tensor("xT", [D, BLb], F32R, kind="ExternalInput")
    wq = nc.dram_tensor("wq", [D, DHC], F32R, kind="ExternalInput")
    wk = nc.dram_tensor("wk", [D, DHC], F32R, kind="ExternalInput")
    wv = nc.dram_tensor("wv", [D, DHC], F32R, kind="ExternalInput")
    wo = nc.dram_tensor("wo", [DHC, D], F32R, kind="ExternalInput")
    bq = nc.dram_tensor("bq", [DHC, 1], F32, kind="ExternalInput")
    bk = nc.dram_tensor("bk", [DHC, 1], F32, kind="ExternalInput")
    bv = nc.dram_tensor("bv", [DHC, 1], F32, kind="ExternalInput")
    out = nc.dram_tensor("out", [BLb, D], F32, kind="ExternalOutput")

    xT_v = xT.ap().rearrange("(kt p) l -> p kt l", p=128)   # [128, KT, BLb]
    wq_v = wq.ap().rearrange("(kt p) m -> p kt m", p=128)   # [128, KT, DHC]
    wk_v = wk.ap().rearrange("(kt p) m -> p kt m", p=128)
    wv_v = wv.ap().rearrange("(kt p) m -> p kt m", p=128)

    with tile.TileContext(nc) as tc, ExitStack() as ctx:
        # --- pools ---
        persist = ctx.enter_context(tc.tile_pool(name="persist", bufs=1))
        xpool = ctx.enter_context(tc.tile_pool(name="xchunk", bufs=2))
        vstage = ctx.enter_context(tc.tile_pool(name="vstage", bufs=2))
        expool = ctx.enter_context(tc.tile_pool(name="expool", bufs=3))
        denpool = ctx.enter_context(tc.tile_pool(name="denpool", bufs=2))
        outpool = ctx.enter_context(tc.tile_pool(name="outpool", bufs=3))
        # PSUM budget: "pair" = 2 banks/tile x3 + "single" = 1 bank/tile x2 -> 8
        scpool = ctx.enter_context(tc.tile_pool(name="scpool", bufs=2, space="PSUM"))
        accpool = ctx.enter_context(tc.tile_pool(name="accpool", bufs=1, space="PSUM"))
        psing = ctx.enter_context(tc.tile_pool(name="psing", bufs=2, space="PSUM"))

        # --- persistent tiles ---
        qT_sb = persist.tile([128, BLb], F32R, tag="qT")
        kT_sb = persist.tile([128, BLb], F32R, tag="kT")
        v_sb = persist.tile([128, B * NJT * VB], F32R, tag="v")
        aT_sb = [
            persist.tile([128, Lb], F32R, tag=f"aT{b}", name=f"aT{b}")
            for b in range(B)
        ]
        wq_sb = persist.tile([128, KT, DHC], F32R, tag="wq")
        wk_sb = persist.tile([128, KT, DHC], F32R, tag="wk")
        wv_sb = persist.tile([128, KT, DHC], F32R, tag="wv")
        wo_sb = persist.tile([DHC, D], F32R, tag="wo")
        bq_sb = persist.tile([DHC, 1], F32, tag="bq")
        bk_sb = persist.tile([DHC, 1], F32, tag="bk")
        bv_sb = persist.tile([DHC, 1], F32, tag="bv")
        ident = persist.tile([128, 128], F32, tag="ident")
        ones64 = persist.tile([65, 64], F32, tag="ones64")  # row 64 used

        # --- phase A: loads & constants ---
        nc.sync.dma_start(wq_sb[:], wq_v)
        nc.sync.dma_start(wk_sb[:], wk_v)
        nc.sync.dma_start(wv_sb[:], wv_v)
        nc.sync.dma_start(wo_sb[:], wo.ap())
        nc.sync.dma_start(bq_sb[:], bq.ap())
        nc.sync.dma_start(bk_sb[:], bk.ap())
        nc.sync.dma_start(bv_sb[:], bv.ap())
        make_identity(nc, ident[:])
        nc.vector.memset(ones64[:], 1.0)
        # fill the aug ones-columns of v (memset can't produce f32r)
        nblk = B * NJT * HPC
        ones_f = persist.tile([128, nblk], F32, tag="ones_f")
        nc.vector.memset(ones_f[:], 1.0)
        v_cols = v_sb[:].rearrange("p (n c) -> p n c", c=65)
        nc.vector.tensor_copy(
            v_cols[:, :, 64:65], ones_f[:].rearrange("p (n c) -> p n c", c=1)
        )

        # --- phase B: projections (chunked over B*L) + v transpose ---
        for chn in range(NPC):
            cs = chn * PC
            xt = xpool.tile([128, KT, PC], F32R, tag="xt")
            nc.sync.dma_start(xt[:], xT_v[:, :, cs:cs + PC])

            for w_sb, b_sb, dst in ((wq_sb, bq_sb, qT_sb), (wk_sb, bk_sb, kT_sb)):
                ps = psing.tile([128, PC], F32, tag="single")
                for kt in range(KT):
                    nc.tensor.matmul(
                        ps[:, :], w_sb[:, kt, :], xt[:, kt, :],
                        start=(kt == 0), stop=(kt == KT - 1),
                    )
                nc.vector.tensor_scalar(
                    dst[:, cs:cs + PC], ps[:, :], b_sb[:, 0:1], None, ALU.add
                )

            # v chunk -> staging (vT layout), then transpose into v_sb blocks
            ps = psing.tile([128, PC], F32, tag="single")
            for kt in range(KT):
                nc.tensor.matmul(
                    ps[:, :], wv_sb[:, kt, :], xt[:, kt, :],
                    start=(kt == 0), stop=(kt == KT - 1),
                )
            vt = vstage.tile([128, PC], F32R, tag="vt")
            nc.vector.tensor_scalar(vt[:], ps[:, :], bv_sb[:, 0:1], None, ALU.add)

            for jl in range(PC // 128):
                gl = cs + jl * 128          # global row offset in [0, BLb)
                b_idx, jt = gl // Lb, (gl % Lb) // 128
                blk = (b_idx * NJT + jt) * VB
                pt = psing.tile([128, 128], F32, tag="single")
                # one [128,128] transpose covers both heads: out[j, h*64+d]
                nc.tensor.transpose(
                    pt[:, :],
                    vt[:, jl * 128:(jl + 1) * 128].bitcast(F32),
                    ident[:],
                )
                vdst = v_sb[:, blk:blk + VB].rearrange("p (h c) -> p h c", h=2)
                ptv = pt[:].rearrange("p (h c) -> p h c", h=2)
                nc.vector.tensor_copy(vdst[:, :, 0:DH], ptv[:, :, :])

        # --- phase C: attention per (batch, query-chunk) ---
        for b in range(B):
            for lc in range(NLC):
                q0 = b * Lb + lc * LC
                aT = accpool.tile([65, 2, LC], F32, tag="acc", padded_shape=[65, 2, 512])
                for jt in range(NJT):
                    k0 = b * Lb + jt * 128
                    blk = (b * NJT + jt) * VB
                    sc = scpool.tile([128, 2, LC], F32, tag="sc", padded_shape=[128, 2, 512])
                    for h in range(HPC):
                        nc.tensor.matmul(
                            sc[:, h, :],
                            kT_sb[h * DH:(h + 1) * DH, k0:k0 + 128],
                            qT_sb[h * DH:(h + 1) * DH, q0:q0 + LC],
                            start=True, stop=True,
                            tile_position=(h * DH, 0) if use_tilepos else None,
                        )
                    ex = expool.tile([128, 2, LC], F32R, tag="ex")
                    nc.scalar.activation(ex[:], sc[:], AF.Exp)
                    for h in range(HPC):
                        nc.tensor.matmul(
                            aT[:, h, :],
                            v_sb[:, blk + h * 65:blk + h * 65 + 65],
                            ex[:, h, :],
                            start=(jt == 0), stop=(jt == NJT - 1),
                        )
                # drain: copy heads into aT_sb; denominators -> reciprocal
                for h in range(HPC):
                    nc.vector.tensor_copy(
                        aT_sb[b][h * DH:(h + 1) * DH, lc * LC:lc * LC + LC],
                        aT[0:DH, h, :],
                    )
                den = denpool.tile([65, 2, LC], F32, tag="den")
                nc.vector.tensor_copy(den[64:65, :, :], aT[64:65, :, :])
                if use_rep:
                    # broadcast raw denominators across partitions, then ONE
                    # wide reciprocal (128 lanes) instead of a 1-lane recip
                    rep = psing.tile([128, LC], F32, tag="single")
                    for h in range(HPC):
                        nc.tensor.matmul(
                            rep[h * DH:(h + 1) * DH, :],
                            ones64[64:65, :],
                            den[64:65, h, :],
                            start=True, stop=True,
                            tile_position=(64, h * DH) if use_tilepos else None,
                        )
                    rrec = denpool.tile([128, LC], F32, tag="rrec")
                    nc.vector.reciprocal(rrec[:, :], rep[:, :])
                    nc.vector.tensor_mul(
                        aT_sb[b][:, lc * LC:lc * LC + LC],
                        aT_sb[b][:, lc * LC:lc * LC + LC],
                        rrec[:, :],
                    )
                # output projection for this query chunk
                for t in range(lc * LC // 128, (lc * LC + LC) // 128):
                    ot = outpool.tile([128, D], F32, tag="ot")
                    for nch in range(2):
                        po = psing.tile([128, 512], F32, tag="single")
                        nc.tensor.matmul(
                            po[:, :],
                            aT_sb[b][:, t * 128:(t + 1) * 128],
                            wo_sb[:, nch * 512:(nch + 1) * 512],
                            start=True, stop=True,
                        )
                        nc.vector.tensor_copy(
                            ot[:, nch * 512:(nch + 1) * 512], po[:, :]
                        )
                    nc.sync.dma_start(
                        out.ap()[b * Lb + t * 128:b * Lb + (t + 1) * 128, :], ot[:]
                    )


    nc.compile()
    return nc


_NC_CACHE = {}


def _get_nc(Lb=L):
    if Lb not in _NC_CACHE:
        _NC_CACHE[Lb] = build(Lb)
    return _NC_CACHE[Lb]


def make_in_maps(x, Wq, bq, Wk, bk, Wv, bv, Wo, bo, Lb=L):
    s = np.float32(DH ** (-0.25))
    BLb = B * Lb
    xT = np.ascontiguousarray(
        np.asarray(x, np.float32).reshape(BLb, D).T
    ).astype(np.float32)
    Wq, Wk, Wv, Wo = (np.asarray(a, np.float32) for a in (Wq, Wk, Wv, Wo))
    bq, bk, bv = (np.asarray(a, np.float32) for a in (bq, bk, bv))
    in_maps = []
    for c in range(NCORES):
        hs = slice(c * DHC, (c + 1) * DHC)
        in_maps.append({
            "xT": xT,
            "wq": np.ascontiguousarray(Wq[:, hs] * s),
            "wk": np.ascontiguousarray(Wk[:, hs] * s),
            "wv": np.ascontiguousarray(Wv[:, hs]),
            "wo": np.ascontiguousarray(Wo[hs, :]),
            "bq": np.ascontiguousarray((bq[hs] * s).reshape(DHC, 1)),
            "bk": np.ascontiguousarray((bk[hs] * s).reshape(DHC, 1)),
            "bv": np.ascontiguousarray(bv[hs].reshape(DHC, 1)),
        })
    return in_maps


def kernel(x, Wq, bq, Wk, bk, Wv, bv, Wo, bo, **run_kwargs):
    x = np.asarray(x, np.float32)
    nc = _get_nc(L)
    in_maps = make_in_maps(x, Wq, bq, Wk, bk, Wv, bv, Wo, bo, L)
    res = bass_utils.run_bass_kernel_spmd(nc, in_maps, list(range(NCORES)), **run_kwargs)
    acc = np.zeros((B * L, D), np.float32)
    for r in res.results:
        acc += r["out"]
    acc += np.asarray(bo, np.float32)[None, :]
    out = acc.reshape(B, L, D)
    kernel.last_results = res
    return out
